# revision 1
# baseline (speedup 1.0000x reference)
"""Trainium2 Bass kernel for nn_CombinedRepeatCausalLinear (PE version).

Math: out[r, t] = sum_{s<=t} x[r, s] * (w0[s]*dv0^(t-s) + w1[t]*dv1^(t-s)) + bias[t]

Chunked linear-attention formulation (chunk L=126 along S):
  - Diagonal blocks D_c[s_l, t_l] (upper-triangular, [128,128] with 2 extra
    "reduction" columns producing decay-weighted chunk sums S0_c, S1_c in
    psum rows 126/127).
  - Cross-chunk contribution is rank-2 per source chunk:
      out[t] += sum_{c'<c(t)} dv0^(t-e_c')*S1_c'[r] + w1[t]*dv1^(t-e_c')*S0_c'[r]
    implemented as a second matmul per chunk against a host-built matrix.

All matmuls are K=128 (host matrices zero-padded) so the PE array stays
fully active and the HAM clock-gate holds the warm 2.4 GHz state; fp32
throughout (HI/LO 2-pass, exact fp32 products). The output is computed
transposed (t on partitions) so the host-built matrices are always the
stationary operand; the host ships x pre-transposed per shard and
transposes the gathered result back.

Data-parallel across 8 NeuronCores on the fused B*E axis.
"""

import sys

if "/opt/trn_rl_repo" not in sys.path:
    sys.path.insert(0, "/opt/trn_rl_repo")

import numpy as np

import concourse.mybir as mybir
from concourse import bacc
from concourse.bass_utils import run_bass_kernel_spmd
from concourse.mybir import AluOpType
from concourse.tile import TileContext

_P = 128
_B, _E, _S = 4, 2048, 2048
_NCORES = 8
_R = (_B * _E) // _NCORES  # 1024 rows (r) per core
_L = 126  # chunk length along S
_NCH = (_S + _L - 1) // _L  # 17 chunks (last has 32)
_HALF = 512  # r per matmul (one PSUM bank, fp32)
_NH = _R // _HALF  # 2 halves

_F32 = mybir.dt.float32


def _chunk_len(c):
    return min(_L, _S - c * _L)


def _build_host_mats(w0, w1, dv0, dv1, bias, with_bias):
    """Build D [128, NCH*128] and M [128, NCH*128] in float64, cast f32."""
    w0 = w0.astype(np.float64)
    w1 = w1.astype(np.float64)
    D = np.zeros((_NCH * _P, _P), dtype=np.float64)
    for c in range(_NCH):
        Lc = _chunk_len(c)
        base = c * _L
        sl = np.arange(Lc)
        tl = np.arange(Lc)
        diff = tl[None, :] - sl[:, None]
        mask = diff >= 0
        blk = np.where(
            mask,
            w0[base + sl][:, None] * (dv0 ** np.maximum(diff, 0))
            + w1[base + tl][None, :] * (dv1 ** np.maximum(diff, 0)),
            0.0,
        )
        Db = D[c * _P : (c + 1) * _P]
        Db[:Lc, :Lc] = blk
        # reduction columns: col 126 -> S0_c (dv1-weighted sum),
        #                    col 127 -> S1_c (w0*dv0-weighted sum)
        Db[:Lc, 126] = dv1 ** (Lc - 1 - sl)
        Db[:Lc, 127] = w0[base + sl] * dv0 ** (Lc - 1 - sl)

    off = 1 if with_bias else 0
    # M padded to 128 contraction rows (rows >= off+2*NCH are zero)
    M = np.zeros((_P, _NCH * _P), dtype=np.float64)
    for c in range(_NCH):
        Lc = _chunk_len(c)
        t = c * _L + np.arange(Lc)
        if with_bias:
            M[0, c * _P : c * _P + Lc] = bias.astype(np.float64)[t]
        for cp in range(c):
            e_cp = cp * _L + _chunk_len(cp) - 1
            M[off + 2 * cp, c * _P : c * _P + Lc] = w1[t] * (dv1 ** (t - e_cp))
            M[off + 2 * cp + 1, c * _P : c * _P + Lc] = dv0 ** (t - e_cp)
    return D.astype(np.float32), M.astype(np.float32)


def _build(with_bias):
    off = 1 if with_bias else 0
    nc = bacc.Bacc(
        "TRN2",
        target_bir_lowering=False,
        debug=False,
        enable_asserts=False,
        num_devices=_NCORES,
    )
    xt = nc.dram_tensor("xt", [_S, _R], _F32, kind="ExternalInput").ap()
    Dd = nc.dram_tensor("Dd", [_NCH * _P, _P], _F32, kind="ExternalInput").ap()
    Md = nc.dram_tensor("Md", [_P, _NCH * _P], _F32, kind="ExternalInput").ap()
    outT = nc.dram_tensor("outT", [_S, _R], _F32, kind="ExternalOutput").ap()

    with TileContext(nc) as tc:
        with (
            tc.tile_pool(name="consts", bufs=1) as cpool,
            tc.tile_pool(name="xin", bufs=8) as xpool,
            tc.tile_pool(name="dg", bufs=1) as dgpool,
            tc.tile_pool(name="ot", bufs=4) as otpool,
            tc.tile_pool(name="pd", bufs=5, space="PSUM") as pdpool,
            tc.tile_pool(name="po", bufs=3, space="PSUM") as popool,
        ):
            sall = cpool.tile([_P, _R], _F32)
            nc.gpsimd.memset(sall[:], 0.0)
            if with_bias:
                nc.gpsimd.memset(sall[0:1, :], 1.0)
            # dedicated last-chunk x tile, zero-filled up front so the
            # memset is off the phase-1 critical path (K=128 contraction
            # reads the zero tail rows)
            xlast = cpool.tile([_P, _R], _F32)
            nc.gpsimd.memset(xlast[:], 0.0)

            # ---- Phase 1: diagonal blocks + chunk reductions ----
            Dt = None
            Mt = None
            dg_tiles = []
            for c in range(_NCH):
                Lc = _chunk_len(c)
                rows = min(_P, _S - c * _L)  # 128, except last chunk: 32
                if rows < _P:
                    xtile = xlast
                else:
                    xtile = xpool.tile([_P, _R], _F32, tag="x", name="x")
                nc.sync.dma_start(xtile[:rows, :], xt[c * _L : c * _L + rows, :])
                dtile = xpool.tile([_P, _P], _F32, tag="d", name="d")
                nc.scalar.dma_start(dtile[:], Dd[c * _P : (c + 1) * _P, :])
                if c == 8:
                    # M is only needed for phase 2; load it mid-phase
                    Mt = cpool.tile([_P, _NCH * _P], _F32)
                    nc.scalar.dma_start(Mt[:], Md[:])
                dg = dgpool.tile([_P, _R], _F32, tag=f"dg{c}", name="dg")
                for h in range(_NH):
                    pd = pdpool.tile([_P, _HALF], _F32, tag="pd", name="pd")
                    nc.tensor.matmul(
                        pd[:],
                        dtile[:],
                        xtile[:, h * _HALF : (h + 1) * _HALF],
                        start=True,
                        stop=True,
                    )
                    nc.vector.tensor_copy(dg[:, h * _HALF : (h + 1) * _HALF], pd[:])
                    # move the chunk-sum rows into Sall partitions (2c, 2c+1)
                    nc.gpsimd.dma_start(
                        sall[off + 2 * c : off + 2 * c + 2, h * _HALF : (h + 1) * _HALF],
                        dg[126:128, h * _HALF : (h + 1) * _HALF],
                    )
                dg_tiles.append(dg)

            # ---- Phase 2: cross-chunk offsets + combine + store ----
            for c in range(_NCH):
                Lc = _chunk_len(c)
                dg = dg_tiles[c]
                if c == 0 and not with_bias:
                    nc.sync.dma_start(outT[0:_L, :], dg[:_L, :])
                    continue
                ot = otpool.tile([_P, _R], _F32, tag="ot", name="ot")
                for h in range(_NH):
                    po = popool.tile([_P, _HALF], _F32, tag="po", name="po")
                    nc.tensor.matmul(
                        po[:],
                        Mt[:, c * _P : (c + 1) * _P],
                        sall[:, h * _HALF : (h + 1) * _HALF],
                        start=True,
                        stop=True,
                    )
                    nc.vector.tensor_tensor(
                        ot[:, h * _HALF : (h + 1) * _HALF],
                        dg[:, h * _HALF : (h + 1) * _HALF],
                        po[:],
                        AluOpType.add,
                    )
                eng = nc.sync if c % 2 == 0 else nc.scalar
                eng.dma_start(outT[c * _L : c * _L + Lc, :], ot[:Lc, :])
    nc.compile()
    return nc


def _run(x, weight, bias, decay_value, trace=False):
    x = np.asarray(x, dtype=np.float32)
    w = np.asarray(weight, dtype=np.float32)
    b = np.asarray(bias, dtype=np.float32)
    dv = np.asarray(decay_value, dtype=np.float32)
    dv0 = float(np.clip(dv[0, 0], 0.9, 1.0))
    dv1 = float(np.clip(dv[1, 0], 0.9, 1.0))
    with_bias = bool(np.any(b))

    D, M = _build_host_mats(w[0], w[1], dv0, dv1, b, with_bias)
    nc = _build(with_bias)

    xf = x.reshape(_B * _E, _S)
    xT = np.ascontiguousarray(xf.T)  # [S, B*E]
    in_maps = []
    for c in range(_NCORES):
        in_maps.append(
            {
                "xt": np.ascontiguousarray(xT[:, c * _R : (c + 1) * _R]),
                "Dd": D,
                "Md": M,
            }
        )

    res = run_bass_kernel_spmd(nc, in_maps, core_ids=list(range(_NCORES)), trace=trace)
    outT = np.concatenate(
        [res.results[c]["outT"] for c in range(_NCORES)], axis=1
    )  # [S, B*E]
    full = np.ascontiguousarray(outT.T).reshape(_B, _E, _S)
    return full, res


def kernel(x, weight, bias, decay_value):
    full, _ = _run(x, weight, bias, decay_value, trace=False)
    return full



# revision 5
# speedup vs baseline: 1.0883x; 1.0883x over previous
"""Trainium2 Bass kernel for nn_CombinedRepeatCausalLinear (folded-scan bf16).

Math: out[r, t] = sum_{s<=t} x[r, s] * (w0[s]*dv0^(t-s) + w1[t]*dv1^(t-s)) + bias[t]

Single-matmul-per-chunk linear-attention scan (chunk L=120 along S):
each chunk's [128,128] stationary D_c folds together
  - the upper-triangular diagonal block,
  - the cross-chunk rank-2 correction, injected via scan-state rows
    carried in the *moving* x tile: A1 (plain decayed sum) and A0
    (w0-weighted decayed sum), each as a bf16 hi+lo pair so the running
    state keeps ~fp32 precision,
  - the new state, produced in 4 psum rows.

DVE partial-partition ops must be quadrant-aligned (4 banks x 32
channels; <=32-row ops may shift the write by whole quadrants), so the
state lives at quadrant offsets 0..3:
  x-tile rows 96..99 = hi block [A1h, A1h, A0h, A0h]
  x-tile rows 64..67 = lo block [A1l, A1l, A0l, A0l]
  data rows: 0..63 -> s_l 0..63, 68..95 -> s_l 64..91, 100..127 -> s_l 92..119
  psum cols 96..99 = [A1_new, A1_new, A0_new, A0_new]
  psum output cols: t<96 -> t, t>=96 -> t+4
Scan hand-off per chunk half:
  v1: xn[96:100] = bf16(pd[96:100])            (hi)
  v2: xn[64:68]  = bf16(pd[96:100] - xn[96:100]) (lo residual, quadrant-
      shifted write)

Everything on the device is bf16 (exact products, fp32 psum
accumulation): 1 cycle/row matmuls and half the HBM traffic of fp32.
The host ships x pre-transposed per shard in bf16, upcasts the bf16
result to fp32, adds bias, and transposes back.

Data-parallel across 8 NeuronCores on the fused B*E axis.
"""

import sys

if "/opt/trn_rl_repo" not in sys.path:
    sys.path.insert(0, "/opt/trn_rl_repo")

import ml_dtypes
import numpy as np

import concourse.mybir as mybir
from concourse import bacc
from concourse.bass_utils import run_bass_kernel_spmd
from concourse.mybir import AluOpType
from concourse.tile import TileContext

_P = 128
_B, _E, _S = 4, 2048, 2048
_NCORES = 8
_R = (_B * _E) // _NCORES  # 1024 rows (r) per core
_L = 120  # data rows per chunk along S
_NCH = (_S + _L - 1) // _L  # 18 chunks (last has 8)
_HALF = 512  # r per matmul (one PSUM bank, fp32)
_NH = _R // _HALF  # 2 halves

_BF16 = mybir.dt.bfloat16
_F32 = mybir.dt.float32
_npbf16 = np.dtype(ml_dtypes.bfloat16)

# x-tile row layout: logical source index s (0..119) -> physical row
_HI = 96  # hi state rows 96..99
_LO = 64  # lo state rows 64..67


def _row_of_s(s):
    return s + (0 if s < 64 else 4) + (0 if s < 92 else 4)


def _col_of_t(t):
    return t if t < 96 else t + 4


def _chunk_len(c):
    return min(_L, _S - c * _L)


def _build_host_d(w0, w1, dv0, dv1):
    """Build D [128, NCH*128] in float64, cast bf16 (stationary per chunk)."""
    w0 = w0.astype(np.float64)
    w1 = w1.astype(np.float64)
    rows = np.array([_row_of_s(s) for s in range(_L)])
    cols = np.array([_col_of_t(t) for t in range(_L)])
    D = np.zeros((_P, _NCH * _P), dtype=np.float64)
    for c in range(_NCH):
        Lc = _chunk_len(c)
        base = c * _L
        blk = D[:, c * _P : (c + 1) * _P]
        sl = np.arange(Lc)
        tl = np.arange(Lc)
        diff = tl[None, :] - sl[:, None]
        mask = diff >= 0
        e = np.maximum(diff, 0)
        blk[np.ix_(rows[:Lc], cols[:Lc])] = np.where(
            mask,
            w0[base + sl][:, None] * (dv0**e) + w1[base + tl][None, :] * (dv1**e),
            0.0,
        )
        if c > 0:
            # state injection: hi+lo pairs get identical coefficients
            blk[_HI, cols[:Lc]] = blk[_LO, cols[:Lc]] = w1[base + tl] * dv1 ** (tl + 1)
            blk[_HI + 2, cols[:Lc]] = blk[_LO + 2, cols[:Lc]] = dv0 ** (tl + 1)
        # new-state producer columns 96..99 = [A1, A1, A0, A0]
        blk[rows[:Lc], 96] = blk[rows[:Lc], 97] = dv1 ** (Lc - 1 - sl)
        blk[rows[:Lc], 98] = blk[rows[:Lc], 99] = w0[base + sl] * dv0 ** (Lc - 1 - sl)
        blk[_HI, 96] = blk[_HI, 97] = dv1**Lc
        blk[_LO, 96] = blk[_LO, 97] = dv1**Lc
        blk[_HI + 2, 98] = blk[_HI + 2, 99] = dv0**Lc
        blk[_LO + 2, 98] = blk[_LO + 2, 99] = dv0**Lc
    return D.astype(_npbf16)


def _build():
    nc = bacc.Bacc(
        "TRN2",
        target_bir_lowering=False,
        debug=False,
        enable_asserts=False,
        num_devices=_NCORES,
    )
    xt = nc.dram_tensor("xt", [_S, _R], _BF16, kind="ExternalInput").ap()
    Dd = nc.dram_tensor("Dd", [_P, _NCH * _P], _BF16, kind="ExternalInput").ap()
    outT = nc.dram_tensor("outT", [_S, _R], _BF16, kind="ExternalOutput").ap()

    with TileContext(nc) as tc:
        with (
            tc.tile_pool(name="consts", bufs=1) as cpool,
            tc.tile_pool(name="xin", bufs=_NCH) as xpool,
            tc.tile_pool(name="ot", bufs=6) as otpool,
            tc.tile_pool(name="pd", bufs=6, space="PSUM") as pdpool,
        ):
            Dall = cpool.tile([_P, _NCH * _P], _BF16)
            nc.scalar.dma_start(Dall[:], Dd[:])

            # allocate all x tiles up front; fire every load immediately so
            # the input queue streams at full rate from t=0
            xtiles = [
                xpool.tile([_P, _R], _BF16, tag="x", name=f"x{c}")
                for c in range(_NCH)
            ]
            # initial scan state is zero (rows 64..127 incl. state blocks;
            # data rows in that range are overwritten by the DMA below)
            nc.vector.memset(xtiles[0][64:128, :], 0.0)
            # last chunk has only 8 data rows; zero everything else
            nc.vector.memset(xtiles[_NCH - 1][:], 0.0)
            for c in range(_NCH):
                Lc = _chunk_len(c)
                base = c * _L
                t = xtiles[c]
                n0 = min(Lc, 64)
                nc.sync.dma_start(t[0:n0, :], xt[base : base + n0, :])
                if Lc > 64:
                    n1 = min(Lc, 92) - 64
                    nc.sync.dma_start(t[68 : 68 + n1, :], xt[base + 64 : base + 64 + n1, :])
                if Lc > 92:
                    n2 = Lc - 92
                    nc.sync.dma_start(t[100 : 100 + n2, :], xt[base + 92 : base + 92 + n2, :])

            for c in range(_NCH):
                Lc = _chunk_len(c)
                base = c * _L
                ot = otpool.tile([_P, _R], _BF16, tag="ot", name="ot")
                for h in range(_NH):
                    cols = slice(h * _HALF, (h + 1) * _HALF)
                    pd = pdpool.tile([_P, _HALF], _F32, tag="pd", name="pd")
                    nc.tensor.matmul(
                        pd[:],
                        Dall[:, c * _P : (c + 1) * _P],
                        xtiles[c][:, cols],
                        start=True,
                        stop=True,
                    )
                    if c + 1 < _NCH:
                        xn = xtiles[c + 1]
                        # scan state hand-off (critical path): hi then lo
                        nc.vector.tensor_copy(xn[96:100, cols], pd[96:100, :])
                        nc.vector.tensor_tensor(
                            xn[64:68, cols],
                            pd[96:100, :],
                            xn[96:100, cols],
                            AluOpType.subtract,
                        )
                    # output downcast: split across vector/scalar engines
                    if h == 0:
                        nc.vector.tensor_copy(ot[:, cols], pd[:])
                    else:
                        nc.scalar.copy(ot[:, cols], pd[:])
                n0 = min(Lc, 96)
                nc.gpsimd.dma_start(outT[base : base + n0, :], ot[0:n0, :])
                if Lc > 96:
                    nc.gpsimd.dma_start(
                        outT[base + 96 : base + Lc, :], ot[100 : 100 + Lc - 96, :]
                    )
    nc.compile()
    return nc


def _run(x, weight, bias, decay_value, trace=False):
    x = np.asarray(x, dtype=np.float32)
    w = np.asarray(weight, dtype=np.float32)
    b = np.asarray(bias, dtype=np.float32)
    dv = np.asarray(decay_value, dtype=np.float32)
    dv0 = float(np.clip(dv[0, 0], 0.9, 1.0))
    dv1 = float(np.clip(dv[1, 0], 0.9, 1.0))

    D = _build_host_d(w[0], w[1], dv0, dv1)
    nc = _build()

    xf = x.reshape(_B * _E, _S)
    xT = xf.T.astype(_npbf16)  # [S, B*E]
    in_maps = []
    for c in range(_NCORES):
        in_maps.append(
            {
                "xt": np.ascontiguousarray(xT[:, c * _R : (c + 1) * _R]),
                "Dd": D,
            }
        )

    res = run_bass_kernel_spmd(nc, in_maps, core_ids=list(range(_NCORES)), trace=trace)
    outT = np.concatenate(
        [np.asarray(res.results[c]["outT"]) for c in range(_NCORES)], axis=1
    )  # [S, B*E] bf16
    full = np.ascontiguousarray(outT.T).astype(np.float32)
    if np.any(b):
        full += b[None, :]
    return full.reshape(_B, _E, _S), res


def kernel(x, weight, bias, decay_value):
    full, _ = _run(x, weight, bias, decay_value, trace=False)
    return full


# revision 12
# speedup vs baseline: 1.7532x; 1.6110x over previous
"""Trainium2 Bass kernel for nn_CombinedRepeatCausalLinear (two-phase bf16).

Math: out[r, t] = sum_{s<=t} x[r, s] * (w0[s]*dv0^(t-s) + w1[t]*dv1^(t-s)) + bias[t]

Chunked linear attention with a matmul-computed scan (chunk L=128, 16
chunks, no padding):

  Phase A -- per-chunk decayed sums U1_c (plain) / U0_c (w0-weighted),
  produced by 15 matmuls that accumulate into ONE psum bank per half
  (stationary places chunk c's sums at partitions 2c, 2c+1).

  Scan -- one matmul per half against a 128x128 triangular decay matrix
  turns the sums into per-chunk exclusive prefix states A1(c), A0(c).

  Phase B -- per chunk: diagonal matmul + rank-2 cross matmul
  (contracting the state rows) accumulated in the same psum bank, then
  one psum->sbuf downcast copy and one output DMA.

There is no serial chunk-to-chunk dependency anywhere: the only scan
"round trip" is sums->bf16->scan-matmul->bf16, ~4 vector/scalar ops
total. Sums and states are kept as bf16 hi+lo pairs (lo = f32 - hi,
computed with quadrant-aligned DVE ops: hi rows 0..31, lo rows 64..95)
so the running state keeps ~fp32 precision.

Everything on the device is bf16 (exact products, fp32 psum
accumulation): 1 cycle/row matmuls and half the HBM traffic of fp32.
The host ships x pre-transposed per shard in bf16, upcasts the bf16
result to fp32, adds bias, and transposes back.

Data-parallel across 8 NeuronCores on the fused B*E axis.
"""

import sys

if "/opt/trn_rl_repo" not in sys.path:
    sys.path.insert(0, "/opt/trn_rl_repo")

import ml_dtypes
import numpy as np

import concourse.mybir as mybir
from concourse import bacc
from concourse.bass_utils import run_bass_kernel_spmd
from concourse.mybir import AluOpType
from concourse.tile import TileContext

_P = 128
_B, _E, _S = 4, 2048, 2048
_NCORES = 8
_R = (_B * _E) // _NCORES  # 1024 rows (r) per core
_L = 128  # chunk length along S
_NCH = _S // _L  # 16 chunks, exact
_HALF = 512  # r per matmul (one PSUM bank, fp32)
_NH = _R // _HALF  # 2 halves

_BF16 = mybir.dt.bfloat16
_F32 = mybir.dt.float32
_npbf16 = np.dtype(ml_dtypes.bfloat16)


def _build_host_mats(w0, w1, dv0, dv1):
    """Build Dall/DS/T/Md in float64, cast bf16."""
    w0 = w0.astype(np.float64)
    w1 = w1.astype(np.float64)
    sl = np.arange(_L)
    tl = np.arange(_L)
    diff = tl[None, :] - sl[:, None]
    mask = diff >= 0
    e = np.maximum(diff, 0)
    Dall = np.zeros((_P, _NCH * _P))
    DS = np.zeros((_P, 2 * _NCH))
    T = np.zeros((_P, _P))
    Md = np.zeros((_P, _NCH * _P))
    for c in range(_NCH):
        base = c * _L
        Dall[:, c * _P : (c + 1) * _P] = np.where(
            mask,
            w0[base + sl][:, None] * (dv0**e) + w1[base + tl][None, :] * (dv1**e),
            0.0,
        )
        DS[:, 2 * c] = dv1 ** (_L - 1 - sl)
        DS[:, 2 * c + 1] = w0[base + sl] * dv0 ** (_L - 1 - sl)
        for cp in range(c):
            d1 = dv1 ** ((c - 1 - cp) * _L)
            d0 = dv0 ** ((c - 1 - cp) * _L)
            T[2 * cp, 2 * c] = T[64 + 2 * cp, 2 * c] = d1
            T[2 * cp + 1, 2 * c + 1] = T[65 + 2 * cp, 2 * c + 1] = d0
        Md[2 * c, c * _P + tl] = Md[64 + 2 * c, c * _P + tl] = w1[base + tl] * dv1 ** (
            tl + 1
        )
        Md[2 * c + 1, c * _P + tl] = Md[65 + 2 * c, c * _P + tl] = dv0 ** (tl + 1)
    cast = lambda a: a.astype(_npbf16)
    return cast(Dall), cast(DS), cast(T), cast(Md)


def _build():
    nc = bacc.Bacc(
        "TRN2",
        target_bir_lowering=False,
        debug=False,
        enable_asserts=False,
        num_devices=_NCORES,
    )
    xt = nc.dram_tensor("xt", [_S, _R], _BF16, kind="ExternalInput").ap()
    Dd = nc.dram_tensor("Dd", [_P, _NCH * _P], _BF16, kind="ExternalInput").ap()
    DSd = nc.dram_tensor("DSd", [_P, 2 * _NCH], _BF16, kind="ExternalInput").ap()
    Td = nc.dram_tensor("Td", [_P, _P], _BF16, kind="ExternalInput").ap()
    Md = nc.dram_tensor("Md", [_P, _NCH * _P], _BF16, kind="ExternalInput").ap()
    outT = nc.dram_tensor("outT", [_S, _R], _BF16, kind="ExternalOutput").ap()

    with TileContext(nc) as tc:
        with (
            tc.tile_pool(name="consts", bufs=1) as cpool,
            tc.tile_pool(name="xin", bufs=_NCH) as xpool,
            tc.tile_pool(name="ot", bufs=6) as otpool,
            tc.tile_pool(name="pacc", bufs=2, space="PSUM") as pspool,
            tc.tile_pool(name="pd", bufs=4, space="PSUM") as pdpool,
        ):
            Dall = cpool.tile([_P, _NCH * _P], _BF16)
            nc.scalar.dma_start(Dall[:], Dd[:])
            Mall = cpool.tile([_P, _NCH * _P], _BF16)
            nc.scalar.dma_start(Mall[:], Md[:])
            Tt = cpool.tile([_P, _P], _BF16)
            nc.scalar.dma_start(Tt[:], Td[:])
            DSt = cpool.tile([_P, 2 * _NCH], _BF16)
            nc.scalar.dma_start(DSt[:], DSd[:])

            # sum stationaries: zero tile with chunk c's two columns placed
            # at free offset c*128 + 2c (partitions 2c, 2c+1 of the psum)
            Dsum = cpool.tile([_P, _NCH * _P], _BF16)
            nc.gpsimd.memset(Dsum[:], 0.0)
            for c in range(_NCH - 1):  # last chunk's sum is never used
                nc.vector.tensor_copy(
                    Dsum[:, c * _P + 2 * c : c * _P + 2 * c + 2],
                    DSt[:, 2 * c : 2 * c + 2],
                )

            sums = cpool.tile([_P, _R], _BF16)
            nc.gpsimd.memset(sums[:], 0.0)
            states = cpool.tile([_P, _R], _BF16)
            nc.gpsimd.memset(states[:], 0.0)

            xtiles = [
                xpool.tile([_P, _R], _BF16, tag="x", name=f"x{c}")
                for c in range(_NCH)
            ]
            for c in range(_NCH):
                nc.sync.dma_start(xtiles[c][:], xt[c * _L : (c + 1) * _L, :])

            # ---- Phase A: chunk sums, accumulated into one bank per half ----
            psums = [
                pspool.tile([_P, _HALF], _F32, tag="acc", name=f"ps{h}")
                for h in range(_NH)
            ]
            for c in range(_NCH - 1):
                for h in range(_NH):
                    nc.tensor.matmul(
                        psums[h][:],
                        Dsum[:, c * _P : (c + 1) * _P],
                        xtiles[c][:, h * _HALF : (h + 1) * _HALF],
                        start=(c == 0),
                        stop=(c == _NCH - 2),
                        skip_group_check=True,
                    )
            for h in range(_NH):
                cols = slice(h * _HALF, (h + 1) * _HALF)
                nc.scalar.copy(sums[0:32, cols], psums[h][0:32, :])
                nc.vector.tensor_tensor(
                    sums[64:96, cols],
                    psums[h][0:32, :],
                    sums[0:32, cols],
                    AluOpType.subtract,
                )

            # ---- Scan: sums -> exclusive prefix states, one matmul/half ----
            for h in range(_NH):
                cols = slice(h * _HALF, (h + 1) * _HALF)
                pst = pspool.tile([_P, _HALF], _F32, tag="acc", name="pst")
                nc.tensor.matmul(
                    pst[:], Tt[:], sums[:, cols], start=True, stop=True
                )
                nc.scalar.copy(states[0:32, cols], pst[0:32, :])
                nc.vector.tensor_tensor(
                    states[64:96, cols],
                    pst[0:32, :],
                    states[0:32, cols],
                    AluOpType.subtract,
                )

            # ---- Phase B: diagonal + cross, psum-accumulated ----
            for c in range(_NCH):
                ot = otpool.tile([_P, _R], _BF16, tag="ot", name="ot")
                for h in range(_NH):
                    cols = slice(h * _HALF, (h + 1) * _HALF)
                    pd = pdpool.tile([_P, _HALF], _F32, tag="pd", name="pd")
                    nc.tensor.matmul(
                        pd[:],
                        Dall[:, c * _P : (c + 1) * _P],
                        xtiles[c][:, cols],
                        start=True,
                        stop=(c == 0),
                    )
                    if c > 0:
                        nc.tensor.matmul(
                            pd[:],
                            Mall[:, c * _P : (c + 1) * _P],
                            states[:, cols],
                            start=False,
                            stop=True,
                        )
                    if h == 0:
                        nc.vector.tensor_copy(ot[:, cols], pd[:])
                    else:
                        nc.scalar.copy(ot[:, cols], pd[:])
                nc.gpsimd.dma_start(outT[c * _L : (c + 1) * _L, :], ot[:])
    nc.compile()
    return nc


def _run(x, weight, bias, decay_value, trace=False):
    x = np.asarray(x, dtype=np.float32)
    w = np.asarray(weight, dtype=np.float32)
    b = np.asarray(bias, dtype=np.float32)
    dv = np.asarray(decay_value, dtype=np.float32)
    dv0 = float(np.clip(dv[0, 0], 0.9, 1.0))
    dv1 = float(np.clip(dv[1, 0], 0.9, 1.0))

    Dall, DS, T, Md = _build_host_mats(w[0], w[1], dv0, dv1)
    nc = _build()

    xf = x.reshape(_B * _E, _S)
    xT = xf.T.astype(_npbf16)  # [S, B*E]
    in_maps = []
    for c in range(_NCORES):
        in_maps.append(
            {
                "xt": np.ascontiguousarray(xT[:, c * _R : (c + 1) * _R]),
                "Dd": Dall,
                "DSd": DS,
                "Td": T,
                "Md": Md,
            }
        )

    res = run_bass_kernel_spmd(nc, in_maps, core_ids=list(range(_NCORES)), trace=trace)
    outT = np.concatenate(
        [np.asarray(res.results[c]["outT"]) for c in range(_NCORES)], axis=1
    )  # [S, B*E] bf16
    full = np.ascontiguousarray(outT.T).astype(np.float32)
    if np.any(b):
        full += b[None, :]
    return full.reshape(_B, _E, _S), res


def kernel(x, weight, bias, decay_value):
    full, _ = _run(x, weight, bias, decay_value, trace=False)
    return full
